# revision 47
# baseline (speedup 1.0000x reference)
"""Trainium2 Bass kernel for nn_CFNOTF_80066780332643.

The reference computes, per 16x16 patch p (flattened to 256 elems):
    y = Re(ifft(fft(p) @ (Wr+iWi) + (br-bi)+i(br+bi)))          [16 ch]
followed by a 3x3 depthwise conv (SAME) on the 256x256 patch grid,
inference BatchNorm, and nearest-resize 256->128 (which picks odd
rows/cols).  The fft->dense->ifft chain is linear in p, so it collapses
to a single real matmul  y = p @ M + d  with
    M = Re(E @ (Wr+iWi) @ G),  d = Re(c @ G)
(E = forward DFT matrix, G = inverse DFT matrix).  BN folds into a
per-channel scale/bias after the conv.

Per-core layout (8 cores, data-parallel over b*patch-rows):
  core c gets image c//4, patch rows (c%4)*64 .. +64, plus a 16-row halo
  below (zeros + mask at the image bottom).  Each 128-pixel-row group is
  one [128, 4096] SBUF tile: partitions = (hi 8, ki 16), free = (kj 16,
  wi 256) -- the host pre-transposes each pixel row to (kj, wi) order so
  every stage-1 moving AP is stride-1 (strided moving runs the PE ~3.5x
  slower).  x streams in fp8 e3m4 (rel err ~1.1e-2 vs the 2e-2 gate,
  and half the HBM traffic of bf16); weights / y / out are bf16.

  y[(hi,oc), wi] accumulates over 16 matmuls (one per kj, block-diag
  bf16 stationary) into one of 4 rotating PSUM banks -- 4 tiles in
  flight hides the ~2.5us evac-semaphore round trip.  The DVE evacuates
  each tile with +d bias into a q-blocked y layout (even/odd wi
  separated) so the conv moving APs are also stride-1.  The 3x3 conv +
  BN + odd-subsample runs as 3 accumulation chains (groups 0-3 / 4-5 /
  6-7, one PSUM bank each) interleaved into the stage-1 stream so only
  the last 6 conv matmuls trail the final group; BN part C runs on the
  scalar engine (Identity activation with scale/bias APs) so the last
  output DMA issues with no cross-engine hop.  Consts ride ahead of x
  on the same queue; the 4 per-partition const vectors ship as 4 DRAM
  rows and are transposed on-chip by a tiny exact f32 matmul (a [128,4]
  DMA would cost 128 single-packet descriptors).
"""

import os
import sys

import numpy as np
from ml_dtypes import bfloat16 as np_mm_dtype
from ml_dtypes import float8_e3m4 as np_x_dtype

for _p in ("/opt/trn_rl_repo", "/root/.axon_site/_ro/trn_rl_repo"):
    if os.path.isdir(_p) and _p not in sys.path:
        sys.path.append(_p)

K = 16
D = 16
EPS = 1e-5
B, H, W = 2, 4096, 4096
HK, WK = H // K, W // K          # 256, 256
N_CORES = 8
ROWS_PER_CORE = HK // 4          # 64 patch rows per core (4 cores / image)
GROUPS = 8                       # 8 full groups of 8 patch rows
GROUP_PIX = 128                  # pixel rows per full group
SLAB_ROWS = GROUPS * GROUP_PIX + K           # 1040 rows: 1024 + 16-row halo
YCOLS = 264                      # per-group y slot: 2 q-blocks of 132 (128 + 4 pad)
OUT_R = 4                        # output rows per group (odd rows 1,3,5,7)

MM_DTYPE = "bfloat16"            # weights / y / output dtype
X_DTYPE = "float8e3"             # x stream dtype (e3m4: rel err 1.1e-2, gate 2e-2)

# packed fp16 weights tile column layout
COL_W2 = 0                       # [128, 2048] stage-2 stationaries
COL_LM = 2048                    # [128, 192] conv main stationaries
COL_LH = 2240                    # [16, 192] conv halo stationaries
COL_MT = 2432                    # [128, 2x16] halo-tile stationaries (M halves)
CST16_W = 2464
# f32 per-partition vectors: shipped as 4 DRAM rows (one packet each,
# plus an I4 block) and transposed on-chip by a tiny f32 matmul -- a
# [128,4] DMA would cost 128 tiny packets (~2us of DMA-engine slots).
CV_DV = 0                        # [128] d bias
CV_MK = 1                        # [16] halo mask
CV_SV = 2                        # [64] BN scale
CV_BV = 3                        # [64] BN bias
CSTV_W = 132                     # 128 values + I4 identity block

LAST_RESULT = None               # BassKernelResults of the last run


def _build_consts(Wr, br, Wi, bi, dw_kernel, dw_bias, gamma, beta,
                  moving_mean, moving_var):
    """Host-side: collapse fft/dense/ifft to M3 [ki,kj,oc], d [oc]; BN scale/bias."""
    Wr = Wr.astype(np.float64)
    Wi = Wi.astype(np.float64)
    fin = K * K
    m = np.arange(fin)
    E = np.exp(-2j * np.pi * np.outer(m, m) / fin)
    n = np.arange(D)
    G = np.exp(2j * np.pi * np.outer(n, n) / D) / D
    M = np.real(E @ (Wr + 1j * Wi) @ G)                  # (256, 16)
    d = np.real(((br - bi) + 1j * (br + bi)).astype(np.complex128) @ G)  # (16,)
    M3 = M.reshape(K, K, D)                              # [ki, kj, oc]

    a_vec = (gamma.astype(np.float64)
             / np.sqrt(moving_var.astype(np.float64) + EPS))
    bias_vec = a_vec * (dw_bias.astype(np.float64)
                        - moving_mean.astype(np.float64)) + beta.astype(np.float64)

    # stage-2 stationaries: per kj a [128,128] block-diag (8 blocks of
    # M3[:, kj, :]) so the hi (patch-row-in-group) axis is preserved.
    W2 = np.zeros((128, K, 128), np.float32)
    for hi in range(8):
        W2[hi * 16:hi * 16 + 16, :, hi * 16:hi * 16 + 16] = \
            M3.transpose(0, 1, 2).astype(np.float32)  # [ki, kj, oc]
    W2 = W2.reshape(128, K * 128)

    # conv stationaries: main [3(dw), 128, 64] with dh taps as bands,
    # halo [3(dw), 16, 64] for the row-8 (next group row 0) tap.
    dwk = dw_kernel[..., 0].astype(np.float32)           # (3, 3, 16) [dh, dw, oc]
    Lmain = np.zeros((3, 128, 64), np.float32)
    Lhalo = np.zeros((3, 16, 64), np.float32)
    for dwi in range(3):
        for r in range(OUT_R):
            for dhi in range(3):
                hi = 2 * r + dhi            # tap row = 2r+1 + (dhi-1)
                for oc in range(D):
                    if hi < 8:
                        Lmain[dwi, hi * 16 + oc, r * 16 + oc] = dwk[dhi, dwi, oc]
                    else:
                        Lhalo[dwi, oc, r * 16 + oc] = dwk[dhi, dwi, oc]

    dvec = np.tile(d.astype(np.float32), 8)[:, None]          # [128, 1]
    scalev = np.tile(a_vec.astype(np.float32), OUT_R)[:, None]   # [64, 1]
    bvec = np.tile(bias_vec.astype(np.float32), OUT_R)[:, None]  # [64, 1]
    return W2, Lmain, Lhalo, dvec, scalev, bvec, M.astype(np.float32)


def _build_nc():
    import concourse.bass as bass
    from concourse import mybir

    mmdt = getattr(mybir.dt, MM_DTYPE)
    xdt = getattr(mybir.dt, X_DTYPE)
    f32 = mybir.dt.float32

    nc = bass.Bass()
    # x stream layout: pairs of groups packed per partition row (8KB
    # packets -- the DMA engines are packet-rate-bound, so 4KB fp8 rows
    # would halve stream bandwidth).  g6/g7 stay separate 4KB-packet DMAs
    # to keep the tail fine-grained.
    xin = nc.declare_dram_parameter("xin", [512, 2 * W], xdt, isOutput=False)
    xhalo = nc.declare_dram_parameter("xhalo", [128, 512], xdt, isOutput=False)
    cst = nc.declare_dram_parameter("cst", [128, CST16_W], mmdt, isOutput=False)
    cstv = nc.declare_dram_parameter("cstv", [4, CSTV_W], f32, isOutput=False)
    out_d = nc.declare_dram_parameter("out", [64, GROUPS * 128], mmdt, isOutput=True)

    ct = nc.alloc_sbuf_tensor("ct", [128, CST16_W], mmdt)
    ctv_in = nc.alloc_sbuf_tensor("ctv_in", [4, CSTV_W], f32)
    ctv = nc.alloc_sbuf_tensor("ctv", [128, 4], f32)
    xg0 = nc.alloc_sbuf_tensor("xg0", [128, W], xdt)
    xg1 = nc.alloc_sbuf_tensor("xg1", [128, W], xdt)
    xps = [nc.alloc_sbuf_tensor(f"xp{q}", [128, 2 * W], xdt)
           for q in range(2)]
    xg6 = nc.alloc_sbuf_tensor("xg6", [128, W], xdt)
    xg7 = nc.alloc_sbuf_tensor("xg7", [128, W], xdt)
    xh = nc.alloc_sbuf_tensor("xh", [128, 512], xdt)

    def x_tile(gsl, pg):
        if gsl == GROUPS:
            return xh[0:pg, :]
        if gsl < 2:
            return (xg0 if gsl == 0 else xg1)[0:pg, :]
        if gsl < 6:
            q = (gsl - 2) // 2
            return xps[q][0:pg, (gsl % 2) * W:(gsl % 2) * W + W]
        return (xg6 if gsl == 6 else xg7)[0:pg, :]
    y_all = nc.alloc_sbuf_tensor("y_all", [128, 9 * YCOLS], mmdt)
    out_sb = nc.alloc_sbuf_tensor("out_sb", [128, GROUPS * 128], mmdt)
    # yp: two half-bank accumulation regions in two separate banks (bank
    # exclusivity vs the DVE evacuation read -- PE-W + DVE-R on one bank
    # is a hardware fault).  convp: two banks (one per 4-group half).
    # yp: four stage-1 accumulation regions in four banks so the PE can
    # run 4 tiles ahead of the DVE evacuation (the evac semaphore round
    # trip is ~2.5us).  convp: banks for conv parts A/B/C; its last bank
    # also hosts the tiny const-transpose scratch (tp).
    yp = nc.alloc_psum_tensor("yp", [128, 2048], f32)
    convp = nc.alloc_psum_tensor("convp", [128, 2048], f32)

    ctap = ct[:]
    cvap = ctv[:]
    dvt = cvap[:, CV_DV:CV_DV + 1]
    mkt = cvap[0:16, CV_MK:CV_MK + 1]
    svt = cvap[0:64, CV_SV:CV_SV + 1]
    bvt = cvap[0:64, CV_BV:CV_BV + 1]
    yav = y_all[:]
    # y layout per group slot: q-blocked [q:2][s:132] so the conv moving
    # APs are stride-1 (col c of y lives at q=c%2, s=c//2; s=128 pad=0).
    yb = yav.rearrange("p (g q s) -> p g q s", g=9, q=2, s=YCOLS // 2)
    osb = out_sb[:].rearrange("p (g s) -> p g s", g=GROUPS)
    # conv PSUM: part A (groups 0-5) in g-slots 0-5 (banks 0-1), part B
    # (groups 6-7) in g-slots 8-9 (bank 2) so BN-A reads never share a
    # bank with PE writes of part B.
    cvv = convp[0:64, :].rearrange("p (g s) -> p g s", g=16)
    tp = convp[:, 1536:1540]

    # stage-1 process order: g0..g6, then the halo tile (host-transposed,
    # just 2 matmuls contracting (ki,kj)=256 over two 128-partition
    # halves), then g7.  PSUM bank rotates by position.
    order = list(range(7)) + [GROUPS, 7]

    with (
        nc.Block() as block,
        nc.semaphore("s_cst") as s_cst,
        nc.semaphore("s_cst_b") as s_cst_b,

        nc.semaphore("s_cstv") as s_cstv,
        nc.semaphore("s_tp") as s_tp,

        nc.semaphore("s_pe") as s_pe,
        nc.semaphore("s_evac") as s_evac,
        nc.semaphore("s_conv") as s_conv,
        nc.semaphore("s_out") as s_out,
        nc.semaphore("s_od") as s_od,
        nc.semaphore("s_ms") as s_ms,
    ):
        # one sem per x DMA
        s_g0 = nc.alloc_semaphore(name="s_g0")
        s_g1 = nc.alloc_semaphore(name="s_g1")
        s_xp = [nc.alloc_semaphore(name=f"s_xp{q}") for q in range(2)]
        s_g6 = nc.alloc_semaphore(name="s_g6")
        s_g7a = nc.alloc_semaphore(name="s_g7a")
        s_g7b = nc.alloc_semaphore(name="s_g7b")
        s_xh = nc.alloc_semaphore(name="s_xh")


        # note: the Bass preamble already clears the kernel sem range and
        # pseudo-barriers, so the NEFF is re-entrant without manual clears.

        @block.sync
        def _(sync: bass.BassEngine):
            # stationaries first (they gate the first matmul), then halo,
            # then the 8 groups -- FIFO on the sync queue.  (Splitting the
            # stream over two hw queues was tried and is ~30% slower: the
            # interleaved streams break DRAM page locality.)
            sync.dma_start(out=ctv_in[:], in_=cstv[:]).then_inc(s_cstv, 16)
            sync.dma_start(out=ctap[:, 0:1024], in_=cst[:, 0:1024]) \
                .then_inc(s_cst, 16)
            sync.dma_start(out=xg0[:], in_=xin[0:128, 0:W]) \
                .then_inc(s_g0, 16)
            sync.dma_start(out=ctap[:, 1024:CST16_W],
                           in_=cst[:, 1024:CST16_W]).then_inc(s_cst_b, 16)
            sync.dma_start(out=xg1[:], in_=xin[0:128, W:2 * W]) \
                .then_inc(s_g1, 16)
            for q in (0, 1):
                sync.dma_start(out=xps[q][:],
                               in_=xin[(q + 1) * 128:(q + 2) * 128, :]) \
                    .then_inc(s_xp[q], 16)
            sync.dma_start(out=xg6[:], in_=xin[384:512, 0:W]) \
                .then_inc(s_g6, 16)
            sync.dma_start(out=xh[:], in_=xhalo[:]).then_inc(s_xh, 16)
            # g7 in two column halves (kj 0-7 / 8-15) so its first 8
            # matmuls overlap the second half's DMA
            sync.dma_start(out=xg7[:, 0:W // 2],
                           in_=xin[384:512, W:W + W // 2]) \
                .then_inc(s_g7a, 16)
            sync.dma_start(out=xg7[:, W // 2:W],
                           in_=xin[384:512, W + W // 2:2 * W]) \
                .then_inc(s_g7b, 16)
            # output DMAs ride this queue too: it is warm (the scalar
            # queue would pay a ~1.5us cold descriptor-engine start at
            # the very end of the kernel)
            sync.wait_ge(s_out, 2)
            sync.dma_start(
                out=out_d[0:64, 0:768],
                in_=out_sb[0:64, 0:768],
            ).then_inc(s_od, 16)
            sync.wait_ge(s_out, 3)
            sync.dma_start(
                out=out_d[0:64, 768:1024],
                in_=out_sb[0:64, 768:1024],
            ).then_inc(s_od, 16)
            sync.wait_ge(s_od, 32)

        CONV_TAPS = ((0, 0), (1, 0), (0, 1))   # dwi -> (q block, s offset)
        # conv parts: (y groups g0:g1, cvv psum slots c0:c1).  One matmul
        # output must stay inside a single 512-col psum bank -> <=4 groups
        # per part; part C is kept small since it is on the critical tail.
        CONV_PARTS = ((0, 4, 0), (4, 6, 4), (6, 8, 8))

        def conv_main(tensor, part):
            g0, g1, c0 = CONV_PARTS[part]
            n = g1 - g0
            for dwi in range(3):
                qsel, s0 = CONV_TAPS[dwi]
                tensor.matmul(
                    cvv[:, c0:c0 + n, :],
                    ctap[:, COL_LM + dwi * 64:COL_LM + dwi * 64 + 64],
                    yb[:, g0:g1, qsel, s0:s0 + 128],
                    start=(dwi == 0), stop=False,
                )

        def conv_halo(tensor, part):
            # row-8 taps of groups g0..g1-1 read row 0 of groups g0+1..g1
            g0, g1, c0 = CONV_PARTS[part]
            n = g1 - g0
            mm = None
            for dwi in range(3):
                qsel, s0 = CONV_TAPS[dwi]
                mm = tensor.matmul(
                    cvv[:, c0:c0 + n, :],
                    ctap[0:16, COL_LH + dwi * 64:COL_LH + dwi * 64 + 64],
                    yb[0:16, g0 + 1:g1 + 1, qsel, s0:s0 + 128],
                    start=False, stop=(dwi == 2),
                )
            mm.then_inc(s_conv, 1)

        @block.tensor
        def _(tensor: bass.BassEngine):
            # transpose the 4 per-partition const rows into [128, 4] via a
            # tiny exact f32 matmul against the I4 block
            tensor.wait_ge(s_cstv, 16)
            tensor.matmul(
                tp,
                ctv_in[0:4, 0:128],
                ctv_in[0:4, 128:132],
                start=True, stop=True,
            ).then_inc(s_tp, 1)
            tensor.wait_ge(s_cst, 16)
            for idx, gsl in enumerate(order):
                pg = 128 if gsl < GROUPS else K
                if gsl == GROUPS:
                    tensor.wait_ge(s_xh, 16)
                elif gsl == 0:
                    tensor.wait_ge(s_g0, 16)
                elif gsl == 1:
                    tensor.wait_ge(s_g1, 16)
                elif gsl < 6:
                    tensor.wait_ge(s_xp[(gsl - 2) // 2], 16)
                elif gsl == 6:
                    tensor.wait_ge(s_g6, 16)
                else:
                    tensor.wait_ge(s_g7a, 16)
                second_wait = s_g7b if gsl == GROUPS - 1 else None
                if idx >= 4:
                    # the target psum bank is free once evac(idx-4) is done
                    tensor.wait_ge(s_evac, idx - 3)
                ypg = yp[0:pg, (idx % 4) * 512:(idx % 4) * 512 + WK]
                mm = None
                if gsl == GROUPS:
                    # halo: contract (ki,kj)=256 in two 128-deep matmuls
                    for h in range(2):
                        mm = tensor.matmul(
                            ypg,
                            ctap[:, COL_MT + h * K:COL_MT + h * K + K],
                            xh[:, h * WK:h * WK + WK],
                            start=(h == 0), stop=(h == 1),
                        )
                else:
                    xv = x_tile(gsl, pg).rearrange("p (k w) -> p k w", k=K)
                    for kj in range(K):
                        if kj == K // 2 and second_wait is not None:
                            tensor.wait_ge(second_wait, 16)
                        if kj == 8 and idx == 0:
                            tensor.wait_ge(s_cst_b, 16)
                        mm = tensor.matmul(
                            ypg,
                            ctap[0:pg, COL_W2 + kj * 128:COL_W2 + kj * 128 + pg],
                            xv[:, kj, :],
                            start=(kj == 0),
                            stop=(kj == K - 1),
                        )
                mm.then_inc(s_pe, 1)
                # conv part P main taps need tiles halo,g0..g_{hi-1}; halo
                # taps additionally need g_hi (see CONV_PARTS wait table)
                if idx == 4:          # g4 issued; A needs tiles <= g3
                    tensor.wait_ge(s_evac, 4)
                    conv_main(tensor, 0)
                elif idx == 5:        # g5 issued; A halo needs g4
                    tensor.wait_ge(s_evac, 5)
                    conv_halo(tensor, 0)
                elif idx == 6:        # g6 issued; B needs g4,g5
                    tensor.wait_ge(s_evac, 6)
                    conv_main(tensor, 1)
            # B halo (needs g6) runs inside g7's evac latency window
            tensor.wait_ge(s_evac, 7)
            conv_halo(tensor, 1)
            tensor.wait_ge(s_evac, 9)
            conv_main(tensor, 2)
            conv_halo(tensor, 2)

        def bn_part(vector, part):
            g0, g1, c0 = ((0, 4, 0), (4, 6, 4), (6, 8, 8))[part]
            vector.wait_ge(s_conv, part + 1)
            vector.tensor_scalar(
                osb[0:64, g0:g1, :], cvv[:, c0:c0 + g1 - g0, :],
                svt[:, :], bvt[:, :],
                mybir.AluOpType.mult,
                mybir.AluOpType.add).then_inc(s_out, 1)

        @block.vector
        def _(vector: bass.BassEngine):
            vector.memset(yav.bitcast(f32), 0.0).then_inc(s_ms, 1)
            vector.wait_ge(s_tp, 1)
            vector.tensor_scalar(
                cvap[:, :], tp, 0.0, None,
                mybir.AluOpType.add).then_inc(s_ms, 1)
            for idx, gsl in enumerate(order):
                pg = 128 if gsl < GROUPS else K
                if idx == 0:
                    vector.wait_ge(s_ms, 2)
                vector.wait_ge(s_pe, idx + 1)
                ypq = yp[0:pg, (idx % 4) * 512:(idx % 4) * 512 + WK] \
                    .rearrange("p (s q) -> p q s", q=2)
                if gsl == GROUPS:
                    ts = vector.tensor_scalar(
                        yb[0:pg, gsl, :, 0:128], ypq,
                        dvt[0:pg, :], mkt[:, :],
                        mybir.AluOpType.add, mybir.AluOpType.mult)
                else:
                    ts = vector.tensor_scalar(
                        yb[0:pg, gsl, :, 0:128], ypq,
                        dvt[0:pg, :], None,
                        mybir.AluOpType.add)
                ts.then_inc(s_evac, 1)
                if idx == 7:
                    bn_part(vector, 0)    # conv A closes during g6
            bn_part(vector, 1)            # overlaps conv C on the PE

        @block.scalar
        def _(scalar: bass.BassEngine):
            scalar.wait_ge(s_ms, 2)       # ctv (svt/bvt) built
            scalar.wait_ge(s_conv, 3)
            scalar.activation(
                osb[0:64, 6:8, :], cvv[:, 8:10, :],
                mybir.ActivationFunctionType.Identity,
                bias=bvt[:, :], scale=svt[:, :],
            ).then_inc(s_out, 1)

    nc.finalize()
    return nc


def prepare_in_maps(x, Wr, br, Wi, bi, dw_kernel, dw_bias, gamma, beta,
                    moving_mean, moving_var):
    x = np.ascontiguousarray(np.asarray(x, np.float32))[..., 0]  # (2, 4096, 4096)
    W2, Lmain, Lhalo, dvec, scalev, bvec, M2 = _build_consts(
        np.asarray(Wr), np.asarray(br), np.asarray(Wi), np.asarray(bi),
        np.asarray(dw_kernel), np.asarray(dw_bias), np.asarray(gamma),
        np.asarray(beta), np.asarray(moving_mean), np.asarray(moving_var))
    cst = np.zeros((128, CST16_W), np_mm_dtype)
    cst[:, COL_W2:COL_W2 + K * 128] = W2.astype(np_mm_dtype)
    cst[:, COL_LM:COL_LM + 192] = \
        Lmain.transpose(1, 0, 2).reshape(128, 3 * 64).astype(np_mm_dtype)
    cst[0:16, COL_LH:COL_LH + 192] = \
        Lhalo.transpose(1, 0, 2).reshape(16, 3 * 64).astype(np_mm_dtype)
    cst[:, COL_MT:COL_MT + K] = M2[0:128].astype(np_mm_dtype)
    cst[:, COL_MT + K:COL_MT + 2 * K] = M2[128:256].astype(np_mm_dtype)
    cv = np.zeros((4, CSTV_W), np.float32)
    cv[CV_DV, 0:128] = dvec[:, 0]
    cv[CV_SV, 0:64] = scalev[:, 0]
    cv[CV_BV, 0:64] = bvec[:, 0]
    cv[:, 128:132] = np.eye(4, dtype=np.float32)

    in_maps = []
    for core in range(N_CORES):
        b, quarter = core // 4, core % 4
        r0 = quarter * GROUPS * GROUP_PIX
        slab = np.zeros((SLAB_ROWS, W), np_x_dtype)
        rows = min(SLAB_ROWS, H - r0)
        # columns reordered (wi,kj)->(kj,wi): each kj-plane is a contiguous
        # 256-col run, so the stage-1 matmul moving APs are stride-1.
        slab[:rows] = (x[b, r0:r0 + rows].reshape(rows, WK, K)
                       .transpose(0, 2, 1).reshape(rows, W).astype(np_x_dtype))
        # pack group pairs side by side (partition p row = rows of groups
        # 2q and 2q+1) so the stream DMAs move 8KB per partition; g6/g7
        # ride the same layout but are fetched as separate column slices.
        xin_arr = np.empty((512, 2 * W), np_x_dtype)
        xin_arr[0:128] = np.concatenate([slab[0:128], slab[128:256]], axis=1)
        xin_arr[128:384] = (slab[256:768].reshape(2, 2, 128, W)
                            .transpose(0, 2, 1, 3).reshape(256, 2 * W))
        xin_arr[384:512] = np.concatenate([slab[768:896], slab[896:1024]],
                                          axis=1)
        cvc = cv.copy()
        cvc[CV_MK, 0:16] = 0.0 if quarter == 3 else 1.0
        # halo transposed to [(ki,kj), wi] halves side by side: the slab
        # rows are already (kj,wi)-ordered, so this is a pure reshape
        halo_t = np.ascontiguousarray(slab[1024:1040]).reshape(256, 256)
        in_maps.append({"xin": xin_arr, "cst": cst, "cstv": cvc,
                        "xhalo": np.concatenate([halo_t[0:128],
                                                 halo_t[128:256]], axis=1)})
    return in_maps


def gather(results):
    out = np.zeros((B, 128, 128, D), np.float32)
    for core in range(N_CORES):
        arr = np.asarray(results[core]["out"])[0:64]      # [64, 8*128]
        arr = arr.reshape(OUT_R, D, GROUPS, 128)          # [r, oc, g, s]
        arr = arr.transpose(2, 0, 3, 1).reshape(32, 128, D)
        b, quarter = core // 4, core % 4
        out[b, quarter * 32:quarter * 32 + 32] = arr
    return out


_NC_CACHE = None


def _ensure_ntff_hook():
    """The agent image's `antenv` lacks `axon_hooks`; bass_utils imports it
    unconditionally when trace=True. Shim the module and register the
    ctypes-based NTFF hook from trn_agent_boot if available."""
    try:
        import antenv.axon_hooks  # noqa: F401
        return True
    except ImportError:
        pass
    try:
        import types
        import antenv
        from trn_agent_boot.trn_boot import _ntff_profile_via_ctypes

        mod = types.ModuleType("antenv.axon_hooks")
        state = {"hook": None}
        mod.set_axon_ntff_profile_hook = lambda h: state.__setitem__("hook", h)
        mod.get_axon_ntff_profile_hook = lambda: state["hook"]
        sys.modules["antenv.axon_hooks"] = mod
        antenv.axon_hooks = mod
        so_path = "/opt/axon/libaxon_pjrt.so"
        if os.path.exists(so_path):
            mod.set_axon_ntff_profile_hook(_ntff_profile_via_ctypes(so_path))
        return True
    except Exception:
        return False


def kernel(x, Wr, br, Wi, bi, dw_kernel, dw_bias, gamma, beta,
           moving_mean, moving_var, _trace=None):
    global LAST_RESULT, _NC_CACHE
    from concourse.bass_utils import run_bass_kernel_spmd

    in_maps = prepare_in_maps(x, Wr, br, Wi, bi, dw_kernel, dw_bias, gamma,
                              beta, moving_mean, moving_var)
    if _NC_CACHE is None:
        _NC_CACHE = _build_nc()
    nc = _NC_CACHE

    trace = (os.environ.get("BASS_TRACE", "") not in ("", "0")
             if _trace is None else _trace)
    if trace and not _ensure_ntff_hook():
        trace = False
    res = run_bass_kernel_spmd(nc, in_maps, list(range(N_CORES)), trace=trace)
    LAST_RESULT = res
    return gather(res.results)


if __name__ == "__main__":
    rng = np.random.default_rng(0)
    inputs = {
        "x": rng.standard_normal((B, H, W, 1), np.float32),
        "Wr": rng.standard_normal((256, D), np.float32) / 16,
        "br": rng.standard_normal(D).astype(np.float32) * 0.02,
        "Wi": rng.standard_normal((256, D), np.float32) / 16,
        "bi": rng.standard_normal(D).astype(np.float32) * 0.02,
        "dw_kernel": rng.standard_normal((3, 3, D, 1), np.float32) * 0.1,
        "dw_bias": rng.standard_normal(D).astype(np.float32) * 0.02,
        "gamma": 1 + 0.1 * rng.standard_normal(D).astype(np.float32),
        "beta": 0.1 * rng.standard_normal(D).astype(np.float32),
        "moving_mean": 0.1 * rng.standard_normal(D).astype(np.float32),
        "moving_var": rng.uniform(0.5, 1.5, D).astype(np.float32),
    }
    out = kernel(**inputs)
    print("out", out.shape, out.dtype, float(np.abs(out).max()))



# revision 48
# speedup vs baseline: 1.1598x; 1.1598x over previous
"""Trainium2 Bass kernel for nn_CFNOTF_80066780332643.

The reference computes, per 16x16 patch p (flattened to 256 elems):
    y = Re(ifft(fft(p) @ (Wr+iWi) + (br-bi)+i(br+bi)))          [16 ch]
followed by a 3x3 depthwise conv (SAME) on the 256x256 patch grid,
inference BatchNorm, and nearest-resize 256->128 (which picks odd
rows/cols).  The fft->dense->ifft chain is linear in p, so it collapses
to a single real matmul  y = p @ M + d  with
    M = Re(E @ (Wr+iWi) @ G),  d = Re(c @ G)
(E = forward DFT matrix, G = inverse DFT matrix).  BN folds into a
per-channel scale/bias after the conv.

Per-core layout (8 cores, data-parallel over b*patch-rows):
  core c gets image c//4, patch rows (c%4)*64 .. +64, plus a 16-row halo
  below (zeros + mask at the image bottom).  Each 128-pixel-row group is
  one [128, 4096] SBUF tile: partitions = (hi 8, ki 16), free = (kj 16,
  wi 256) -- the host pre-transposes each pixel row to (kj, wi) order so
  every stage-1 moving AP is stride-1 (strided moving runs the PE ~3.5x
  slower).  x streams in fp8 e3m4 (rel err ~1.1e-2 vs the 2e-2 gate,
  and half the HBM traffic of bf16); weights / y / out are bf16.

  y[(hi,oc), wi] accumulates over 16 matmuls (one per kj, block-diag
  bf16 stationary) into one of 4 rotating PSUM banks -- 4 tiles in
  flight hides the ~2.5us evac-semaphore round trip.  The DVE evacuates
  each tile with +d bias into a q-blocked y layout (even/odd wi
  separated) so the conv moving APs are also stride-1.  The 3x3 conv +
  BN + odd-subsample runs as 3 accumulation chains (groups 0-3 / 4-5 /
  6-7, one PSUM bank each) interleaved into the stage-1 stream so only
  the last 6 conv matmuls trail the final group; BN part C runs on the
  scalar engine (Identity activation with scale/bias APs) so the last
  output DMA issues with no cross-engine hop.  Consts ride ahead of x
  on the same queue; the 4 per-partition const vectors ship as 4 DRAM
  rows and are transposed on-chip by a tiny exact f32 matmul (a [128,4]
  DMA would cost 128 single-packet descriptors).
"""

import os
import sys

import numpy as np
from ml_dtypes import bfloat16 as np_mm_dtype
from ml_dtypes import float8_e3m4 as np_x_dtype

for _p in ("/opt/trn_rl_repo", "/root/.axon_site/_ro/trn_rl_repo"):
    if os.path.isdir(_p) and _p not in sys.path:
        sys.path.append(_p)

K = 16
D = 16
EPS = 1e-5
B, H, W = 2, 4096, 4096
HK, WK = H // K, W // K          # 256, 256
N_CORES = 8
ROWS_PER_CORE = HK // 4          # 64 patch rows per core (4 cores / image)
GROUPS = 8                       # 8 full groups of 8 patch rows
GROUP_PIX = 128                  # pixel rows per full group
SLAB_ROWS = GROUPS * GROUP_PIX + K           # 1040 rows: 1024 + 16-row halo
YCOLS = 264                      # per-group y slot: 2 q-blocks of 132 (128 + 4 pad)
OUT_R = 4                        # output rows per group (odd rows 1,3,5,7)

MM_DTYPE = "bfloat16"            # weights / y / output dtype
X_DTYPE = "float8e3"             # x stream dtype (e3m4: rel err 1.1e-2, gate 2e-2)

# packed fp16 weights tile column layout
COL_W2 = 0                       # [128, 2048] stage-2 stationaries
COL_LM = 2048                    # [128, 192] conv main stationaries
COL_LH = 2240                    # [16, 192] conv halo stationaries
COL_MT = 2432                    # [128, 2x16] halo-tile stationaries (M halves)
CST16_W = 2464
# f32 per-partition vectors: shipped as 4 DRAM rows (one packet each,
# plus an I4 block) and transposed on-chip by a tiny f32 matmul -- a
# [128,4] DMA would cost 128 tiny packets (~2us of DMA-engine slots).
CV_DV = 0                        # [128] d bias
CV_MK = 1                        # [16] halo mask
CV_SV = 2                        # [64] BN scale
CV_BV = 3                        # [64] BN bias
CSTV_W = 132                     # 128 values + I4 identity block

LAST_RESULT = None               # BassKernelResults of the last run


def _build_consts(Wr, br, Wi, bi, dw_kernel, dw_bias, gamma, beta,
                  moving_mean, moving_var):
    """Host-side: collapse fft/dense/ifft to M3 [ki,kj,oc], d [oc]; BN scale/bias."""
    Wr = Wr.astype(np.float64)
    Wi = Wi.astype(np.float64)
    fin = K * K
    m = np.arange(fin)
    E = np.exp(-2j * np.pi * np.outer(m, m) / fin)
    n = np.arange(D)
    G = np.exp(2j * np.pi * np.outer(n, n) / D) / D
    M = np.real(E @ (Wr + 1j * Wi) @ G)                  # (256, 16)
    d = np.real(((br - bi) + 1j * (br + bi)).astype(np.complex128) @ G)  # (16,)
    M3 = M.reshape(K, K, D)                              # [ki, kj, oc]

    a_vec = (gamma.astype(np.float64)
             / np.sqrt(moving_var.astype(np.float64) + EPS))
    bias_vec = a_vec * (dw_bias.astype(np.float64)
                        - moving_mean.astype(np.float64)) + beta.astype(np.float64)

    # stage-2 stationaries: per kj a [128,128] block-diag (8 blocks of
    # M3[:, kj, :]) so the hi (patch-row-in-group) axis is preserved.
    W2 = np.zeros((128, K, 128), np.float32)
    for hi in range(8):
        W2[hi * 16:hi * 16 + 16, :, hi * 16:hi * 16 + 16] = \
            M3.transpose(0, 1, 2).astype(np.float32)  # [ki, kj, oc]
    W2 = W2.reshape(128, K * 128)

    # conv stationaries: main [3(dw), 128, 64] with dh taps as bands,
    # halo [3(dw), 16, 64] for the row-8 (next group row 0) tap.
    dwk = dw_kernel[..., 0].astype(np.float32)           # (3, 3, 16) [dh, dw, oc]
    Lmain = np.zeros((3, 128, 64), np.float32)
    Lhalo = np.zeros((3, 16, 64), np.float32)
    for dwi in range(3):
        for r in range(OUT_R):
            for dhi in range(3):
                hi = 2 * r + dhi            # tap row = 2r+1 + (dhi-1)
                for oc in range(D):
                    if hi < 8:
                        Lmain[dwi, hi * 16 + oc, r * 16 + oc] = dwk[dhi, dwi, oc]
                    else:
                        Lhalo[dwi, oc, r * 16 + oc] = dwk[dhi, dwi, oc]

    dvec = np.tile(d.astype(np.float32), 8)[:, None]          # [128, 1]
    scalev = np.tile(a_vec.astype(np.float32), OUT_R)[:, None]   # [64, 1]
    bvec = np.tile(bias_vec.astype(np.float32), OUT_R)[:, None]  # [64, 1]
    return W2, Lmain, Lhalo, dvec, scalev, bvec, M.astype(np.float32)


def _build_nc():
    import concourse.bass as bass
    from concourse import mybir

    mmdt = getattr(mybir.dt, MM_DTYPE)
    xdt = getattr(mybir.dt, X_DTYPE)
    f32 = mybir.dt.float32

    nc = bass.Bass()
    # x stream layout: pairs of groups packed per partition row (8KB
    # packets -- the DMA engines are packet-rate-bound, so 4KB fp8 rows
    # would halve stream bandwidth).  g6/g7 stay separate 4KB-packet DMAs
    # to keep the tail fine-grained.
    xin = nc.declare_dram_parameter("xin", [512, 2 * W], xdt, isOutput=False)
    xhalo = nc.declare_dram_parameter("xhalo", [128, 512], xdt, isOutput=False)
    cst = nc.declare_dram_parameter("cst", [128, CST16_W], mmdt, isOutput=False)
    cstv = nc.declare_dram_parameter("cstv", [4, CSTV_W], f32, isOutput=False)
    out_d = nc.declare_dram_parameter("out", [64, GROUPS * 128], mmdt, isOutput=True)

    ct = nc.alloc_sbuf_tensor("ct", [128, CST16_W], mmdt)
    ctv_in = nc.alloc_sbuf_tensor("ctv_in", [4, CSTV_W], f32)
    ctv = nc.alloc_sbuf_tensor("ctv", [128, 4], f32)
    xg0 = nc.alloc_sbuf_tensor("xg0", [128, W], xdt)
    xg1 = nc.alloc_sbuf_tensor("xg1", [128, W], xdt)
    xps = [nc.alloc_sbuf_tensor(f"xp{q}", [128, 2 * W], xdt)
           for q in range(2)]
    xg6 = nc.alloc_sbuf_tensor("xg6", [128, W], xdt)
    xg7 = nc.alloc_sbuf_tensor("xg7", [128, W], xdt)
    xh = nc.alloc_sbuf_tensor("xh", [128, 512], xdt)

    def x_tile(gsl, pg):
        if gsl == GROUPS:
            return xh[0:pg, :]
        if gsl < 2:
            return (xg0 if gsl == 0 else xg1)[0:pg, :]
        if gsl < 6:
            q = (gsl - 2) // 2
            return xps[q][0:pg, (gsl % 2) * W:(gsl % 2) * W + W]
        return (xg6 if gsl == 6 else xg7)[0:pg, :]
    y_all = nc.alloc_sbuf_tensor("y_all", [128, 9 * YCOLS], mmdt)
    out_sb = nc.alloc_sbuf_tensor("out_sb", [128, GROUPS * 128], mmdt)
    # yp: two half-bank accumulation regions in two separate banks (bank
    # exclusivity vs the DVE evacuation read -- PE-W + DVE-R on one bank
    # is a hardware fault).  convp: two banks (one per 4-group half).
    # yp: four stage-1 accumulation regions in four banks so the PE can
    # run 4 tiles ahead of the DVE evacuation (the evac semaphore round
    # trip is ~2.5us).  convp: banks for conv parts A/B/C; its last bank
    # also hosts the tiny const-transpose scratch (tp).
    yp = nc.alloc_psum_tensor("yp", [128, 2048], f32)
    convp = nc.alloc_psum_tensor("convp", [128, 2048], f32)

    ctap = ct[:]
    cvap = ctv[:]
    dvt = cvap[:, CV_DV:CV_DV + 1]
    mkt = cvap[0:16, CV_MK:CV_MK + 1]
    svt = cvap[0:64, CV_SV:CV_SV + 1]
    bvt = cvap[0:64, CV_BV:CV_BV + 1]
    yav = y_all[:]
    # y layout per group slot: q-blocked [q:2][s:132] so the conv moving
    # APs are stride-1 (col c of y lives at q=c%2, s=c//2; s=128 pad=0).
    yb = yav.rearrange("p (g q s) -> p g q s", g=9, q=2, s=YCOLS // 2)
    osb = out_sb[:].rearrange("p (g s) -> p g s", g=GROUPS)
    # conv PSUM: part A (groups 0-5) in g-slots 0-5 (banks 0-1), part B
    # (groups 6-7) in g-slots 8-9 (bank 2) so BN-A reads never share a
    # bank with PE writes of part B.
    cvv = convp[0:64, :].rearrange("p (g s) -> p g s", g=16)
    tp = convp[:, 1536:1540]

    # stage-1 process order: g0..g6, then the halo tile (host-transposed,
    # just 2 matmuls contracting (ki,kj)=256 over two 128-partition
    # halves), then g7.  PSUM bank rotates by position.
    order = list(range(7)) + [GROUPS, 7]

    with (
        nc.Block() as block,
        nc.semaphore("s_cst") as s_cst,
        nc.semaphore("s_cst_b") as s_cst_b,

        nc.semaphore("s_cstv") as s_cstv,
        nc.semaphore("s_tp") as s_tp,

        nc.semaphore("s_pe") as s_pe,
        nc.semaphore("s_evac") as s_evac,
        nc.semaphore("s_conv") as s_conv,
        nc.semaphore("s_out") as s_out,
        nc.semaphore("s_od") as s_od,
        nc.semaphore("s_ms") as s_ms,
    ):
        # one sem per x DMA
        s_g0 = nc.alloc_semaphore(name="s_g0")
        s_g1 = nc.alloc_semaphore(name="s_g1")
        s_xp = [nc.alloc_semaphore(name=f"s_xp{q}") for q in range(2)]
        s_g6 = nc.alloc_semaphore(name="s_g6")
        s_g7a = nc.alloc_semaphore(name="s_g7a")
        s_g7b = nc.alloc_semaphore(name="s_g7b")
        s_xh = nc.alloc_semaphore(name="s_xh")


        # note: the Bass preamble already clears the kernel sem range and
        # pseudo-barriers, so the NEFF is re-entrant without manual clears.

        @block.sync
        def _(sync: bass.BassEngine):
            # stationaries first (they gate the first matmul), then halo,
            # then the 8 groups -- FIFO on the sync queue.  (Splitting the
            # stream over two hw queues was tried and is ~30% slower: the
            # interleaved streams break DRAM page locality.)
            sync.dma_start(out=ctv_in[:], in_=cstv[:]).then_inc(s_cstv, 16)
            sync.dma_start(out=ctap[:, 0:1024], in_=cst[:, 0:1024]) \
                .then_inc(s_cst, 16)
            sync.dma_start(out=xg0[:], in_=xin[0:128, 0:W]) \
                .then_inc(s_g0, 16)
            sync.dma_start(out=ctap[:, 1024:CST16_W],
                           in_=cst[:, 1024:CST16_W]).then_inc(s_cst_b, 16)
            sync.dma_start(out=xg1[:], in_=xin[0:128, W:2 * W]) \
                .then_inc(s_g1, 16)
            for q in (0, 1):
                sync.dma_start(out=xps[q][:],
                               in_=xin[(q + 1) * 128:(q + 2) * 128, :]) \
                    .then_inc(s_xp[q], 16)
            sync.dma_start(out=xg6[:], in_=xin[384:512, 0:W]) \
                .then_inc(s_g6, 16)
            sync.dma_start(out=xh[:], in_=xhalo[:]).then_inc(s_xh, 16)
            # g7 in two column halves (kj 0-7 / 8-15) so its first 8
            # matmuls overlap the second half's DMA
            sync.dma_start(out=xg7[:, 0:W // 2],
                           in_=xin[384:512, W:W + W // 2]) \
                .then_inc(s_g7a, 16)
            sync.dma_start(out=xg7[:, W // 2:W],
                           in_=xin[384:512, W + W // 2:2 * W]) \
                .then_inc(s_g7b, 16)

        CONV_TAPS = ((0, 0), (1, 0), (0, 1))   # dwi -> (q block, s offset)
        # conv parts: (y groups g0:g1, cvv psum slots c0:c1).  One matmul
        # output must stay inside a single 512-col psum bank -> <=4 groups
        # per part; part C is kept small since it is on the critical tail.
        CONV_PARTS = ((0, 4, 0), (4, 6, 4), (6, 8, 8))

        def conv_main(tensor, part):
            g0, g1, c0 = CONV_PARTS[part]
            n = g1 - g0
            for dwi in range(3):
                qsel, s0 = CONV_TAPS[dwi]
                tensor.matmul(
                    cvv[:, c0:c0 + n, :],
                    ctap[:, COL_LM + dwi * 64:COL_LM + dwi * 64 + 64],
                    yb[:, g0:g1, qsel, s0:s0 + 128],
                    start=(dwi == 0), stop=False,
                )

        def conv_halo(tensor, part):
            # row-8 taps of groups g0..g1-1 read row 0 of groups g0+1..g1
            g0, g1, c0 = CONV_PARTS[part]
            n = g1 - g0
            mm = None
            for dwi in range(3):
                qsel, s0 = CONV_TAPS[dwi]
                mm = tensor.matmul(
                    cvv[:, c0:c0 + n, :],
                    ctap[0:16, COL_LH + dwi * 64:COL_LH + dwi * 64 + 64],
                    yb[0:16, g0 + 1:g1 + 1, qsel, s0:s0 + 128],
                    start=False, stop=(dwi == 2),
                )
            mm.then_inc(s_conv, 1)

        @block.tensor
        def _(tensor: bass.BassEngine):
            # transpose the 4 per-partition const rows into [128, 4] via a
            # tiny exact f32 matmul against the I4 block
            tensor.wait_ge(s_cstv, 16)
            tensor.matmul(
                tp,
                ctv_in[0:4, 0:128],
                ctv_in[0:4, 128:132],
                start=True, stop=True,
            ).then_inc(s_tp, 1)
            tensor.wait_ge(s_cst, 16)
            for idx, gsl in enumerate(order):
                pg = 128 if gsl < GROUPS else K
                if gsl == GROUPS:
                    tensor.wait_ge(s_xh, 16)
                elif gsl == 0:
                    tensor.wait_ge(s_g0, 16)
                elif gsl == 1:
                    tensor.wait_ge(s_g1, 16)
                elif gsl < 6:
                    tensor.wait_ge(s_xp[(gsl - 2) // 2], 16)
                elif gsl == 6:
                    tensor.wait_ge(s_g6, 16)
                else:
                    tensor.wait_ge(s_g7a, 16)
                second_wait = s_g7b if gsl == GROUPS - 1 else None
                if idx >= 4:
                    # the target psum bank is free once evac(idx-4) is done
                    tensor.wait_ge(s_evac, idx - 3)
                ypg = yp[0:pg, (idx % 4) * 512:(idx % 4) * 512 + WK]
                mm = None
                if gsl == GROUPS:
                    # halo: contract (ki,kj)=256 in two 128-deep matmuls
                    for h in range(2):
                        mm = tensor.matmul(
                            ypg,
                            ctap[:, COL_MT + h * K:COL_MT + h * K + K],
                            xh[:, h * WK:h * WK + WK],
                            start=(h == 0), stop=(h == 1),
                        )
                else:
                    xv = x_tile(gsl, pg).rearrange("p (k w) -> p k w", k=K)
                    for kj in range(K):
                        if kj == K // 2 and second_wait is not None:
                            tensor.wait_ge(second_wait, 16)
                        if kj == 8 and idx == 0:
                            tensor.wait_ge(s_cst_b, 16)
                        mm = tensor.matmul(
                            ypg,
                            ctap[0:pg, COL_W2 + kj * 128:COL_W2 + kj * 128 + pg],
                            xv[:, kj, :],
                            start=(kj == 0),
                            stop=(kj == K - 1),
                        )
                mm.then_inc(s_pe, 1)
                # conv part P main taps need tiles halo,g0..g_{hi-1}; halo
                # taps additionally need g_hi (see CONV_PARTS wait table)
                if idx == 4:          # g4 issued; A needs tiles <= g3
                    tensor.wait_ge(s_evac, 4)
                    conv_main(tensor, 0)
                elif idx == 5:        # g5 issued; A halo needs g4
                    tensor.wait_ge(s_evac, 5)
                    conv_halo(tensor, 0)
                elif idx == 6:        # g6 issued; B needs g4,g5
                    tensor.wait_ge(s_evac, 6)
                    conv_main(tensor, 1)
            # B halo (needs g6) runs inside g7's evac latency window
            tensor.wait_ge(s_evac, 7)
            conv_halo(tensor, 1)
            tensor.wait_ge(s_evac, 9)
            conv_main(tensor, 2)
            conv_halo(tensor, 2)

        def bn_part(vector, part):
            g0, g1, c0 = ((0, 4, 0), (4, 6, 4), (6, 8, 8))[part]
            vector.wait_ge(s_conv, part + 1)
            vector.tensor_scalar(
                osb[0:64, g0:g1, :], cvv[:, c0:c0 + g1 - g0, :],
                svt[:, :], bvt[:, :],
                mybir.AluOpType.mult,
                mybir.AluOpType.add).then_inc(s_out, 1)

        @block.vector
        def _(vector: bass.BassEngine):
            vector.memset(yav.bitcast(f32), 0.0).then_inc(s_ms, 1)
            vector.wait_ge(s_tp, 1)
            vector.tensor_scalar(
                cvap[:, :], tp, 0.0, None,
                mybir.AluOpType.add).then_inc(s_ms, 1)
            for idx, gsl in enumerate(order):
                pg = 128 if gsl < GROUPS else K
                if idx == 0:
                    vector.wait_ge(s_ms, 2)
                vector.wait_ge(s_pe, idx + 1)
                ypq = yp[0:pg, (idx % 4) * 512:(idx % 4) * 512 + WK] \
                    .rearrange("p (s q) -> p q s", q=2)
                if gsl == GROUPS:
                    ts = vector.tensor_scalar(
                        yb[0:pg, gsl, :, 0:128], ypq,
                        dvt[0:pg, :], mkt[:, :],
                        mybir.AluOpType.add, mybir.AluOpType.mult)
                else:
                    ts = vector.tensor_scalar(
                        yb[0:pg, gsl, :, 0:128], ypq,
                        dvt[0:pg, :], None,
                        mybir.AluOpType.add)
                ts.then_inc(s_evac, 1)
                if idx == 7:
                    bn_part(vector, 0)    # conv A closes during g6
            bn_part(vector, 1)            # overlaps conv C on the PE

        @block.scalar
        def _(scalar: bass.BassEngine):
            scalar.wait_ge(s_out, 2)
            scalar.dma_start(
                out=out_d[0:64, 0:768],
                in_=out_sb[0:64, 0:768],
            ).then_inc(s_od, 16)
            scalar.wait_ge(s_ms, 2)       # ctv (svt/bvt) built
            scalar.wait_ge(s_conv, 3)
            scalar.activation(
                osb[0:64, 6:8, :], cvv[:, 8:10, :],
                mybir.ActivationFunctionType.Identity,
                bias=bvt[:, :], scale=svt[:, :],
            ).then_inc(s_out, 1)
            scalar.wait_ge(s_out, 3)
            scalar.dma_start(
                out=out_d[0:64, 768:1024],
                in_=out_sb[0:64, 768:1024],
            ).then_inc(s_od, 16)
            scalar.wait_ge(s_od, 32)

    nc.finalize()
    return nc


def prepare_in_maps(x, Wr, br, Wi, bi, dw_kernel, dw_bias, gamma, beta,
                    moving_mean, moving_var):
    x = np.ascontiguousarray(np.asarray(x, np.float32))[..., 0]  # (2, 4096, 4096)
    W2, Lmain, Lhalo, dvec, scalev, bvec, M2 = _build_consts(
        np.asarray(Wr), np.asarray(br), np.asarray(Wi), np.asarray(bi),
        np.asarray(dw_kernel), np.asarray(dw_bias), np.asarray(gamma),
        np.asarray(beta), np.asarray(moving_mean), np.asarray(moving_var))
    cst = np.zeros((128, CST16_W), np_mm_dtype)
    cst[:, COL_W2:COL_W2 + K * 128] = W2.astype(np_mm_dtype)
    cst[:, COL_LM:COL_LM + 192] = \
        Lmain.transpose(1, 0, 2).reshape(128, 3 * 64).astype(np_mm_dtype)
    cst[0:16, COL_LH:COL_LH + 192] = \
        Lhalo.transpose(1, 0, 2).reshape(16, 3 * 64).astype(np_mm_dtype)
    cst[:, COL_MT:COL_MT + K] = M2[0:128].astype(np_mm_dtype)
    cst[:, COL_MT + K:COL_MT + 2 * K] = M2[128:256].astype(np_mm_dtype)
    cv = np.zeros((4, CSTV_W), np.float32)
    cv[CV_DV, 0:128] = dvec[:, 0]
    cv[CV_SV, 0:64] = scalev[:, 0]
    cv[CV_BV, 0:64] = bvec[:, 0]
    cv[:, 128:132] = np.eye(4, dtype=np.float32)

    in_maps = []
    for core in range(N_CORES):
        b, quarter = core // 4, core % 4
        r0 = quarter * GROUPS * GROUP_PIX
        slab = np.zeros((SLAB_ROWS, W), np_x_dtype)
        rows = min(SLAB_ROWS, H - r0)
        # columns reordered (wi,kj)->(kj,wi): each kj-plane is a contiguous
        # 256-col run, so the stage-1 matmul moving APs are stride-1.
        slab[:rows] = (x[b, r0:r0 + rows].reshape(rows, WK, K)
                       .transpose(0, 2, 1).reshape(rows, W).astype(np_x_dtype))
        # pack group pairs side by side (partition p row = rows of groups
        # 2q and 2q+1) so the stream DMAs move 8KB per partition; g6/g7
        # ride the same layout but are fetched as separate column slices.
        xin_arr = np.empty((512, 2 * W), np_x_dtype)
        xin_arr[0:128] = np.concatenate([slab[0:128], slab[128:256]], axis=1)
        xin_arr[128:384] = (slab[256:768].reshape(2, 2, 128, W)
                            .transpose(0, 2, 1, 3).reshape(256, 2 * W))
        xin_arr[384:512] = np.concatenate([slab[768:896], slab[896:1024]],
                                          axis=1)
        cvc = cv.copy()
        cvc[CV_MK, 0:16] = 0.0 if quarter == 3 else 1.0
        # halo transposed to [(ki,kj), wi] halves side by side: the slab
        # rows are already (kj,wi)-ordered, so this is a pure reshape
        halo_t = np.ascontiguousarray(slab[1024:1040]).reshape(256, 256)
        in_maps.append({"xin": xin_arr, "cst": cst, "cstv": cvc,
                        "xhalo": np.concatenate([halo_t[0:128],
                                                 halo_t[128:256]], axis=1)})
    return in_maps


def gather(results):
    out = np.zeros((B, 128, 128, D), np.float32)
    for core in range(N_CORES):
        arr = np.asarray(results[core]["out"])[0:64]      # [64, 8*128]
        arr = arr.reshape(OUT_R, D, GROUPS, 128)          # [r, oc, g, s]
        arr = arr.transpose(2, 0, 3, 1).reshape(32, 128, D)
        b, quarter = core // 4, core % 4
        out[b, quarter * 32:quarter * 32 + 32] = arr
    return out


_NC_CACHE = None


def _ensure_ntff_hook():
    """The agent image's `antenv` lacks `axon_hooks`; bass_utils imports it
    unconditionally when trace=True. Shim the module and register the
    ctypes-based NTFF hook from trn_agent_boot if available."""
    try:
        import antenv.axon_hooks  # noqa: F401
        return True
    except ImportError:
        pass
    try:
        import types
        import antenv
        from trn_agent_boot.trn_boot import _ntff_profile_via_ctypes

        mod = types.ModuleType("antenv.axon_hooks")
        state = {"hook": None}
        mod.set_axon_ntff_profile_hook = lambda h: state.__setitem__("hook", h)
        mod.get_axon_ntff_profile_hook = lambda: state["hook"]
        sys.modules["antenv.axon_hooks"] = mod
        antenv.axon_hooks = mod
        so_path = "/opt/axon/libaxon_pjrt.so"
        if os.path.exists(so_path):
            mod.set_axon_ntff_profile_hook(_ntff_profile_via_ctypes(so_path))
        return True
    except Exception:
        return False


def kernel(x, Wr, br, Wi, bi, dw_kernel, dw_bias, gamma, beta,
           moving_mean, moving_var, _trace=None):
    global LAST_RESULT, _NC_CACHE
    from concourse.bass_utils import run_bass_kernel_spmd

    in_maps = prepare_in_maps(x, Wr, br, Wi, bi, dw_kernel, dw_bias, gamma,
                              beta, moving_mean, moving_var)
    if _NC_CACHE is None:
        _NC_CACHE = _build_nc()
    nc = _NC_CACHE

    trace = (os.environ.get("BASS_TRACE", "") not in ("", "0")
             if _trace is None else _trace)
    if trace and not _ensure_ntff_hook():
        trace = False
    res = run_bass_kernel_spmd(nc, in_maps, list(range(N_CORES)), trace=trace)
    LAST_RESULT = res
    return gather(res.results)


if __name__ == "__main__":
    rng = np.random.default_rng(0)
    inputs = {
        "x": rng.standard_normal((B, H, W, 1), np.float32),
        "Wr": rng.standard_normal((256, D), np.float32) / 16,
        "br": rng.standard_normal(D).astype(np.float32) * 0.02,
        "Wi": rng.standard_normal((256, D), np.float32) / 16,
        "bi": rng.standard_normal(D).astype(np.float32) * 0.02,
        "dw_kernel": rng.standard_normal((3, 3, D, 1), np.float32) * 0.1,
        "dw_bias": rng.standard_normal(D).astype(np.float32) * 0.02,
        "gamma": 1 + 0.1 * rng.standard_normal(D).astype(np.float32),
        "beta": 0.1 * rng.standard_normal(D).astype(np.float32),
        "moving_mean": 0.1 * rng.standard_normal(D).astype(np.float32),
        "moving_var": rng.uniform(0.5, 1.5, D).astype(np.float32),
    }
    out = kernel(**inputs)
    print("out", out.shape, out.dtype, float(np.abs(out).max()))

